# revision 1
# baseline (speedup 1.0000x reference)
"""Grouped-query causal attention on 8 TRN2 NeuronCores.

Problem: q [B=2, S=2048, H=32, D=128], k/v [B=2, S=2048, HKV=8, D=128],
causal softmax(q k^T / sqrt(D)) v with G = H // HKV = 4 query heads per
kv head.

Sharding (no collectives needed): 8 cores = 2 batches x 4 kv-head-pairs.
Each core computes 8 query heads / 2 kv heads of one batch element.

Per-core kernel design:
  - scores are built TRANSPOSED (S^T[k, q] tiles, k on partitions) so that
    softmax(P^T) feeds the P@V matmul directly as lhsT with no on-chip
    transposes at all.
  - Q^T/K^T [d, s] layouts come from a bf16 DRAM bounce (gpsimd casting
    DMA fp32->bf16) followed by a 2-byte xbar DMA transpose load.
  - exp runs on ScalarE reading PSUM score slabs of 3 banks [128, 1536]
    per instruction, writing bf16 P^T straight to SBUF.
  - softmax denominators ride along the P@V matmul as a ones-column
    appended to V (output column 128 = row sums), so no reductions are
    needed anywhere; VectorE just does reciprocal + scale at the end.
  - causality: per-(k-tile, q-tile) skipping plus one triangular
    -1e9 mask add on diagonal 128x128 blocks.
"""

import numpy as np

_B, _S, _H, _HKV, _D = 2, 2048, 32, 8, 128
_G = _H // _HKV  # 4 query heads per kv head
_NCORES = 8
_SHARDS = 4  # head shards; cores = _B * _SHARDS
_H_PER = _H // _SHARDS  # 8
_KV_PER = _HKV // _SHARDS  # 2

_P = 128  # partition / tile edge
_QB = 512  # q columns per block (4 q tiles)
_KG = 2  # k tiles per PSUM score slab (2 banks)

_build_cache = {}


def build_program(S=_S, n_heads=_H_PER, n_kv=_KV_PER, g=_G):
    """Emit + compile the single-core Tile program (SPMD: same NEFF on all
    cores, only the input data differs)."""
    import concourse.mybir as mybir
    import concourse.tile as tile
    from concourse import bacc
    from concourse.tile import add_dep_helper
    from contextlib import ExitStack

    dt = mybir.dt
    AF = mybir.ActivationFunctionType
    ALU = mybir.AluOpType

    D, P, QB, KG = _D, _P, _QB, _KG
    n_qt = S // P  # 128-row tiles along the sequence
    n_qb = S // QB  # q blocks
    qtb = QB // P  # q tiles per block (4)
    scale = float(D) ** -0.5

    nc = bacc.Bacc("TRN2", target_bir_lowering=False, debug=False)
    q_in = nc.dram_tensor("q", [S, n_heads, D], dt.float32, kind="ExternalInput").ap()
    k_in = nc.dram_tensor("k", [S, n_kv, D], dt.float32, kind="ExternalInput").ap()
    v_in = nc.dram_tensor("v", [S, n_kv, D], dt.float32, kind="ExternalInput").ap()
    o_out = nc.dram_tensor("out", [S, n_heads, D], dt.float32, kind="ExternalOutput").ap()

    with tile.TileContext(nc) as tc, ExitStack() as ctx:
        const_pool = ctx.enter_context(tc.tile_pool(name="const", bufs=1))
        dram_pool = ctx.enter_context(tc.tile_pool(name="bounce", bufs=8, space="DRAM"))
        qt_pool = ctx.enter_context(tc.tile_pool(name="qT", bufs=4))
        kt_pool = ctx.enter_context(tc.tile_pool(name="kT", bufs=2))
        v_pool = ctx.enter_context(tc.tile_pool(name="vv", bufs=2))
        fstg_pool = ctx.enter_context(tc.tile_pool(name="fstg", bufs=3))
        pt_pool = ctx.enter_context(tc.tile_pool(name="pT", bufs=4))
        osb_pool = ctx.enter_context(tc.tile_pool(name="osb", bufs=3))
        rc_pool = ctx.enter_context(tc.tile_pool(name="rc", bufs=8))
        sc_pool = ctx.enter_context(tc.tile_pool(name="sc", bufs=3, space="PSUM"))
        acc_pool = ctx.enter_context(tc.tile_pool(name="acc", bufs=2, space="PSUM"))

        # Causal masking of the diagonal 128x128 block happens AFTER exp, on
        # P^T in SBUF: multiply by a 0/1 triangle (bf16, DVE 2x mode). This
        # keeps VectorE entirely off the QK->exp critical chain, and the
        # masked P^T block feeds the diagonal k-tile's PV matmul, which is
        # naturally the last one of its group.
        tri01 = const_pool.tile([P, P], dt.bfloat16)
        nc.gpsimd.memset(tri01[:], 1.0)
        nc.gpsimd.affine_select(
            out=tri01[:],
            in_=tri01[:],
            pattern=[[1, P]],
            base=0,
            channel_multiplier=-1,
            compare_op=ALU.is_ge,
            fill=0.0,
        )

        def load_xT_swdge(src, pool, tag):
            """ONE SWDGE casting DMA into a bf16 bounce + ONE xbar load."""
            bounce = dram_pool.tile([S, D], dt.bfloat16, tag="bounce", name="bounce")
            nc.gpsimd.dma_start(out=bounce[:], in_=src)
            xT = pool.tile([P, S], dt.bfloat16, tag=tag, name=tag)
            nc.sync.dma_start_transpose(xT[:], bounce[:])
            return xT

        def load_xT_gp(src, pool, tag, chunks=1):
            """HWDGE strided load -> gpsimd compute cast -> HWDGE store ->
            xbar load. Offloads the SWDGE casting queue (which, under
            8-core DMA contention, can barely cover ~6 slabs per kernel).
            chunks>1 pipelines at 512-row granularity for startup latency."""
            xT = pool.tile([P, S], dt.bfloat16, tag=tag, name=tag)
            rows = S // chunks
            rt = rows // P
            for c in range(chunks):
                sl = src[c * rows : (c + 1) * rows, :]
                stg = fstg_pool.tile([P, rt, D], dt.float32, tag="fstg", name="fstg")
                nc.sync.dma_start(out=stg[:], in_=sl.rearrange("(t p) d -> p t d", p=P))
                stgb = fstg_pool.tile([P, rt, D], dt.bfloat16, tag="fstgb", name="fstgb")
                nc.gpsimd.tensor_copy(out=stgb[:], in_=stg[:])
                bounce = dram_pool.tile([rows, D], dt.bfloat16, tag="bounce", name="bounce")
                nc.sync.dma_start(out=bounce[:].rearrange("(t p) d -> p t d", p=P), in_=stgb[:])
                nc.sync.dma_start_transpose(xT[:, c * rows : (c + 1) * rows], bounce[:])
            return xT

        def load_k(kv):
            if kv == 0:
                return load_xT_gp(k_in[:, kv, :], kt_pool, "kT", chunks=4)
            return load_xT_swdge(k_in[:, kv, :], kt_pool, "kT")

        def load_q(h):
            if h == 0:
                return load_xT_gp(q_in[:, h, :], qt_pool, "qT", chunks=4)
            if h in (3, 6):
                return load_xT_gp(q_in[:, h, :], qt_pool, "qT")
            return load_xT_swdge(q_in[:, h, :], qt_pool, "qT")

        def load_v(kv):
            vv = v_pool.tile([P, n_qt, D + 1], dt.bfloat16, tag="vv", name="vv")
            nc.gpsimd.memset(vv[:, :, D], 1.0)
            chunks = 4 if kv == 0 else 1
            rt = n_qt // chunks
            for c in range(chunks):
                vstg = v_pool.tile([P, rt, D], dt.float32, tag="vstg", name="vstg")
                nc.sync.dma_start(
                    out=vstg[:],
                    in_=v_in[c * rt * P : (c + 1) * rt * P, kv, :].rearrange(
                        "(t p) d -> p t d", p=P
                    ),
                )
                nc.gpsimd.tensor_copy(out=vv[:, c * rt : (c + 1) * rt, 0:D], in_=vstg[:])
            return vv

        # prefetched tiles, keyed by head / kv-head index
        kTs, qTs, vvs = {}, {}, {}

        def prefetch(hh):
            if hh >= n_heads:
                return
            hkv = hh // g
            if hkv not in kTs:
                kTs[hkv] = load_k(hkv)
            qTs[hh] = load_q(hh)
            if hkv not in vvs:
                vvs[hkv] = load_v(hkv)

        prefetch(0)
        prefetch(1)

        for h in range(n_heads):
            kv = h // g
            prefetch(h + 2)  # keep load chains ~2 head-windows ahead
            kT = kTs[kv]
            qT = qTs.pop(h)
            vv = vvs[kv]
            if h % g == g - 1:  # last head using this kv group
                del kTs[kv], vvs[kv]

            # flat list of score groups for this head: (qb, g0, g1)
            groups = []
            for qb in range(n_qb):
                n_k = (qb + 1) * qtb
                for g0 in range(0, n_k, KG):
                    groups.append((qb, g0, min(g0 + KG, n_k)))

            accs_of = {}  # qb -> 4 accumulator tiles
            live = {}  # group idx -> pT tile

            def emit_qk(gi, kT=kT, qT=qT, h=h):
                qb, g0, g1 = groups[gi]
                sc = sc_pool.tile([P, KG * QB], dt.float32, tag="sc", name="sc")
                for j in range(g0, g1):
                    jl = j - g0
                    # full-width QK even on diagonal tiles: the non-causal
                    # leading columns become real (unread) exp values, which
                    # lets exp run as ONE activate per group.
                    nc.tensor.matmul(
                        out=sc[:, jl * QB : (jl + 1) * QB],
                        lhsT=kT[:, j * P : (j + 1) * P],
                        rhs=qT[:, qb * QB : (qb + 1) * QB],
                        start=True,
                        stop=True,
                    )
                W = (g1 - g0) * QB
                pT = pt_pool.tile([P, KG * QB], dt.bfloat16, tag="pT", name="pT")
                nc.scalar.activation(out=pT[:, :W], in_=sc[:, :W], func=AF.Exp, scale=scale)
                for j in range(g0, g1):
                    if j >= qb * qtb:  # diagonal tile: zero exp of k>q entries
                        jl = j - g0
                        t = j - qb * qtb
                        blk = pT[:, jl * QB + t * P : jl * QB + (t + 1) * P]
                        nc.vector.tensor_tensor(out=blk, in0=blk, in1=tri01[:], op=ALU.mult)
                live[gi] = pT

            def emit_pv(gi, vv=vv, h=h):
                qb, g0, g1 = groups[gi]
                pT = live.pop(gi)
                if g0 == 0:
                    # two accumulators packed per PSUM bank; region r of a
                    # tile is cols [r*(D+1), (r+1)*(D+1)). Only region 0's
                    # first matmul uses start=True (clears the whole bank's
                    # has_written bits); region 1's first matmul relies on
                    # still-pending bits to overwrite, so it must execute
                    # after region 0's start (manual dep below).
                    accs_of[qb] = [
                        acc_pool.tile([P, 2 * (D + 1)], dt.float32, tag="acc", name=f"accp{r}")
                        for r in range(qtb // 2)
                    ]
                accs = accs_of[qb]
                first_mm = {}
                for j in range(g0, g1):
                    jl = j - g0
                    for it in range(qtb):
                        qt_abs = qb * qtb + it
                        if qt_abs < j:
                            continue  # fully masked block
                        tile_, r = accs[it // 2], it % 2
                        mm = nc.tensor.matmul(
                            out=tile_[:, r * (D + 1) : (r + 1) * (D + 1)],
                            lhsT=pT[:, jl * QB + it * P : jl * QB + (it + 1) * P],
                            rhs=vv[:, j, :],
                            start=(j == 0 and r == 0),
                            stop=(j == qt_abs),
                            skip_group_check=True,
                        )
                        if j == 0:
                            first_mm[it] = mm
                            if r == 1:
                                add_dep_helper(
                                    mm.ins,
                                    first_mm[it - 1].ins,
                                    sync=False,
                                    reason="acc bank-mate ordering (pending-zero)",
                                )
                if g1 == (qb + 1) * qtb:  # last group of this q block: finalize
                    o_sb = osb_pool.tile([P, qtb, D], dt.float32, tag="osb", name="osb")
                    for it in range(qtb):
                        tile_, r = accs[it // 2], it % 2
                        o0 = r * (D + 1)
                        rc = rc_pool.tile([P, 1], dt.float32, tag="rc", name="rc")
                        nc.vector.reciprocal(rc[:], tile_[:, o0 + D : o0 + D + 1])
                        nc.vector.tensor_scalar_mul(o_sb[:, it, :], tile_[:, o0 : o0 + D], rc[:])
                    nc.sync.dma_start(
                        out=o_out[qb * QB : (qb + 1) * QB, h, :].rearrange(
                            "(t p) d -> p t d", p=P
                        ),
                        in_=o_sb[:],
                    )
                    del accs_of[qb]

            # depth-2 software pipeline: keep two QK groups in flight ahead
            # of PV so ACT runs back-to-back and the PE never waits on it.
            emit_qk(0)
            if len(groups) > 1:
                emit_qk(1)
            for gi in range(len(groups)):
                if gi + 2 < len(groups):
                    emit_qk(gi + 2)
                emit_pv(gi)

    nc.compile()
    return nc


def _get_program():
    key = "full"
    if key not in _build_cache:
        _build_cache[key] = build_program()
    return _build_cache[key]


def kernel(q, k, v):
    from concourse import bass_utils

    q = np.ascontiguousarray(np.asarray(q, dtype=np.float32))
    k = np.ascontiguousarray(np.asarray(k, dtype=np.float32))
    v = np.ascontiguousarray(np.asarray(v, dtype=np.float32))
    assert q.shape == (_B, _S, _H, _D), q.shape
    assert k.shape == (_B, _S, _HKV, _D), k.shape

    nc = _get_program()

    in_maps = []
    for c in range(_NCORES):
        b, p = divmod(c, _SHARDS)
        in_maps.append(
            {
                "q": np.ascontiguousarray(q[b, :, p * _H_PER : (p + 1) * _H_PER, :]),
                "k": np.ascontiguousarray(k[b, :, p * _KV_PER : (p + 1) * _KV_PER, :]),
                "v": np.ascontiguousarray(v[b, :, p * _KV_PER : (p + 1) * _KV_PER, :]),
            }
        )

    res = bass_utils.run_bass_kernel_spmd(nc, in_maps, list(range(_NCORES))).results

    out = np.empty((_B, _S, _H, _D), dtype=np.float32)
    for c in range(_NCORES):
        b, p = divmod(c, _SHARDS)
        out[b, :, p * _H_PER : (p + 1) * _H_PER, :] = res[c]["out"]
    return out

